# revision 11
# baseline (speedup 1.0000x reference)
"""MetaMoE Trainium2 kernel: 16 experts sharded 2-per-core across 8 NeuronCores.

Each core computes: shared LayerNorm of x, the (replicated) softmax gate, its two
experts' MLP chains, and the gate-weighted partial sum [B, 2]. The host sums the
8 partials and applies the final mean/var head split.

v2 layout strategy:
- All weights are folded (LN gain into w1) and cast to bf16 on the HOST, packed
  in the exact SBUF layout, so device DMA is contiguous and no on-device
  staging/cast pass is needed.
- x is normalized per 128-row tile, then transposed to feature-major via
  tensor-engine transposes (matmul-with-identity) into PSUM, copied to SBUF by
  the otherwise-idle GpSimd engine. This removes the serialized sync-ring
  DMA-transposes of v1 (and their ordering hazard against other DMA traffic).
- Chunk-pipelined schedule: per 512-batch chunk, LN+transpose -> gate -> expert0
  are emitted together so the tensor engine starts ~15us into the kernel;
  expert1 streams afterward from fully-resident inputs.
"""
import sys
import os

sys.path.insert(0, "/opt/trn_rl_repo")

import numpy as np
import ml_dtypes  # noqa: F401

import concourse.bass as bass  # noqa: F401
import concourse.mybir as mybir
from concourse import bacc
from concourse.tile import TileContext
from concourse.bass_utils import run_bass_kernel_spmd

F32 = mybir.dt.float32
BF16 = mybir.dt.bfloat16
AF = mybir.ActivationFunctionType
ALU = mybir.AluOpType
AX = mybir.AxisListType

B, IN, HID, G1, E = 4096, 1024, 2048, 256, 16
NCORES = 8
EPL = E // NCORES          # experts per core
NB = B // 128              # 32 batch tiles
NK = IN // 128             # 8 contraction tiles for w1 / gate w1
NM = HID // 128            # 16 m-tiles of h1
KH = HID // 128            # 16 contraction tiles for w2
NG = G1 // 128             # 2 m/k tiles for gate hidden
CH = 512                   # batch chunk (matmul moving free dim)
NCH = B // CH              # 8 chunks
BPC = CH // 128            # 4 b-tiles per chunk
EPS = 1e-5


def build_nc():
    nc = bacc.Bacc(None)

    x = nc.dram_tensor("x", [B, IN], BF16, kind="ExternalInput")
    identd = nc.dram_tensor("identd", [128, 128], BF16, kind="ExternalInput")
    w1 = nc.dram_tensor("w1", [EPL * NK * 128, HID], BF16, kind="ExternalInput")
    w2 = nc.dram_tensor("w2", [EPL * KH * 128, G1], BF16, kind="ExternalInput")
    gw1 = nc.dram_tensor("gw1", [128, NK, G1], BF16, kind="ExternalInput")
    gw2 = nc.dram_tensor("gw2", [128, NG, E], BF16, kind="ExternalInput")
    w3 = nc.dram_tensor("w3", [128, 2 * EPL * 2], BF16, kind="ExternalInput")
    gb1 = nc.dram_tensor("gb1", [128, NG], F32, kind="ExternalInput")
    eb1 = nc.dram_tensor("eb1", [128, EPL, NM], F32, kind="ExternalInput")
    eb2 = nc.dram_tensor("eb2", [128, EPL, NG], F32, kind="ExternalInput")
    gb2 = nc.dram_tensor("gb2", [E], F32, kind="ExternalInput")
    eb3 = nc.dram_tensor("eb3", [EPL, 2], F32, kind="ExternalInput")
    out = nc.dram_tensor("out", [B, 2], F32, kind="ExternalOutput")

    with TileContext(nc) as tc:
        with (
            tc.tile_pool(name="cpool", bufs=1) as cpool,
            tc.tile_pool(name="w1pool", bufs=EPL * NK) as w1pool,
            tc.tile_pool(name="w2pool", bufs=EPL * KH) as w2pool,
            tc.tile_pool(name="stage", bufs=2) as stpool,
            tc.tile_pool(name="xtpool", bufs=3) as xtpool,
            tc.tile_pool(name="xnbpool", bufs=8) as xnbpool,
            tc.tile_pool(name="hpool", bufs=1) as hpool,
            tc.tile_pool(name="psT", bufs=2, space="PSUM") as psT,
            tc.tile_pool(name="psA", bufs=2, space="PSUM") as psA,
            tc.tile_pool(name="psB", bufs=2, space="PSUM") as psB,
            tc.tile_pool(name="psC", bufs=2, space="PSUM") as psC,
        ):
            # ---------------- persistent tiles ----------------
            xnT = cpool.tile([128, NK, B], BF16)             # normalized x, transposed
            gw1b = cpool.tile([128, NK, G1], BF16)
            gw2b = cpool.tile([128, NG, E], BF16)
            w3b = cpool.tile([128, 2, EPL, 2], BF16)         # [p, k3, e, t]
            gb1_t = cpool.tile([128, NG], F32)
            b2bc = cpool.tile([128, E], F32)
            b3bc = cpool.tile([128, EPL * 2], F32)
            eb1_t = cpool.tile([128, EPL, NM], F32)
            eb2_t = cpool.tile([128, EPL, NG], F32)
            exp_all = cpool.tile([128, NB, E], F32)
            recip_all = cpool.tile([128, NB], F32)
            acc = cpool.tile([128, NB, 2], F32)
            ident = cpool.tile([128, 128], BF16)

            # Small constants ride the scalar queue so the sync queue's first
            # descriptors are the x tiles the LN pipeline is waiting on.
            nc.scalar.dma_start(ident[:], identd[:, :])
            nc.scalar.dma_start(gw1b[:], gw1[:, :, :])
            nc.scalar.dma_start(gw2b[:], gw2[:, :, :])
            nc.scalar.dma_start(
                w3b[:], w3.rearrange("p (k e t) -> p k e t", k=2, e=EPL))
            W1 = {}
            W2 = {}

            def emit_w1(e, eng):
                for k in range(NK):
                    t = w1pool.tile([128, HID], BF16, tag="w1")
                    r0 = (e * NK + k) * 128
                    eng.dma_start(t[:], w1[r0:r0 + 128, :])
                    W1[(e, k)] = t

            def emit_w2(e, eng):
                for k2 in range(KH):
                    t = w2pool.tile([128, G1], BF16, tag="w2")
                    r0 = (e * KH + k2) * 128
                    eng.dma_start(t[:], w2[r0:r0 + 128, :])
                    W2[(e, k2)] = t

            # ---------------- small constant loads (scalar ring) ---------------
            nc.scalar.dma_start(gb1_t[:], gb1[:, :])
            nc.scalar.dma_start(eb1_t[:], eb1[:, :, :])
            nc.scalar.dma_start(eb2_t[:], eb2[:, :, :])
            b2row = stpool.tile([1, E], F32, tag="b2row")
            nc.scalar.dma_start(b2row[:1, :], gb2[None, :])
            b3row = stpool.tile([1, EPL * 2], F32, tag="b3row")
            nc.scalar.dma_start(b3row[:1, :], eb3.rearrange("e t -> (e t)")[None, :])
            nc.gpsimd.partition_broadcast(b2bc[:], b2row[:1, :])
            nc.gpsimd.partition_broadcast(b3bc[:], b3row[:1, :])

            # ---------------- LayerNorm + tensor-engine transpose ------------
            XNB = {}

            def ln_stage(bt):
                xt = xtpool.tile([128, IN], BF16, tag="xt")
                nc.sync.dma_start(xt[:], x[bt * 128:(bt + 1) * 128, :])
                st6 = stpool.tile([128, 2, 6], F32, tag="st6")
                for g in range(2):
                    nc.vector.bn_stats(st6[:, g, :], xt[:, g * 512:(g + 1) * 512])
                st = stpool.tile([128, 2], F32, tag="st")
                nc.vector.bn_aggr(st[:], st6[:])
                rstd = stpool.tile([128, 1], F32, tag="rstd")
                nc.vector.tensor_scalar_add(rstd[:], st[:, 1:2], EPS)
                nc.scalar.sqrt(rstd[:], rstd[:])
                nc.vector.reciprocal(rstd[:], rstd[:])
                xnb = xnbpool.tile([128, IN], BF16, tag="xnb")
                nc.gpsimd.tensor_scalar(xnb[:], xt[:], st[:, 0:1], rstd[:],
                                        op0=ALU.subtract, op1=ALU.mult)
                XNB[bt] = xnb

            def xp_stage(bt):
                xnb = XNB.pop(bt)
                ps = psT.tile([128, NK, 128], BF16, tag="psT")
                for k in range(NK):
                    nc.tensor.transpose(ps[:, k], xnb[:, k * 128:(k + 1) * 128],
                                        ident[:])
                nc.vector.tensor_copy(xnT[:, :, bt * 128:(bt + 1) * 128], ps[:])

            # ---------------- gate chunk routine ----------------
            def gate_chains(ch):
                c0 = ch * CH
                g1s = stpool.tile([128, NG, CH], BF16, tag="g1s")
                for m in range(NG):
                    ps = psB.tile([128, CH], F32, tag="psB")
                    for k in range(NK):
                        nc.tensor.matmul(ps[:], gw1b[:, k, m * 128:(m + 1) * 128],
                                         xnT[:, k, c0:c0 + CH],
                                         start=(k == 0), stop=(k == NK - 1))
                    nc.scalar.activation(g1s[:, m], ps[:], AF.Relu,
                                         bias=gb1_t[:, m:m + 1])
                return g1s

            def gate_logits(ch, g1s):
                for bl in range(BPC):
                    bt = ch * BPC + bl
                    ps = psC.tile([128, E], F32, tag="psC")
                    for k2 in range(NG):
                        nc.tensor.matmul(ps[:], g1s[:, k2, bl * 128:(bl + 1) * 128],
                                         gw2b[:, k2],
                                         start=(k2 == 0), stop=(k2 == NG - 1))
                    lg = stpool.tile([128, E], F32, tag="lg")
                    nc.vector.tensor_add(lg[:], ps[:], b2bc[:])
                    mx = stpool.tile([128, 1], F32, tag="mx")
                    nc.vector.tensor_reduce(mx[:], lg[:], axis=AX.X, op=ALU.max,
                                            negate=True)
                    sm = stpool.tile([128, 1], F32, tag="sm")
                    nc.scalar.activation(exp_all[:, bt], lg[:], AF.Exp,
                                         bias=mx[:, 0:1], accum_out=sm[:, 0:1])
                    nc.vector.reciprocal(recip_all[:, bt:bt + 1], sm[:])

            # ---------------- expert chunk routine ----------------
            h1s = hpool.tile([128, NM, CH], BF16)

            def w1_stage(e, ch):
                c0 = ch * CH
                for m in range(NM):
                    ps = psA.tile([128, CH], F32, tag="psA")
                    for k in range(NK):
                        nc.tensor.matmul(ps[:], W1[(e, k)][:, m * 128:(m + 1) * 128],
                                         xnT[:, k, c0:c0 + CH],
                                         start=(k == 0), stop=(k == NK - 1))
                    nc.scalar.activation(h1s[:, m], ps[:], AF.Relu,
                                         bias=eb1_t[:, e, m:m + 1])

            def w2_stage(e, ch):
                h2t = stpool.tile([128, NG, CH], BF16, tag="h2s")
                for m2 in range(NG):
                    ps = psB.tile([128, CH], F32, tag="psB")
                    for k2 in range(KH):
                        nc.tensor.matmul(ps[:], W2[(e, k2)][:, m2 * 128:(m2 + 1) * 128],
                                         h1s[:, k2],
                                         start=(k2 == 0), stop=(k2 == KH - 1))
                    nc.scalar.activation(h2t[:, m2], ps[:], AF.Relu,
                                         bias=eb2_t[:, e, m2:m2 + 1])
                return h2t

            def w3_stage(e, ch, h2t):
                for bl in range(BPC):
                    bt = ch * BPC + bl
                    ps = psC.tile([128, E], F32, tag="psC")
                    for k3 in range(2):
                        nc.tensor.matmul(ps[:, :2], h2t[:, k3, bl * 128:(bl + 1) * 128],
                                         w3b[:, k3, e],
                                         start=(k3 == 0), stop=(k3 == 1))
                    eo = stpool.tile([128, 2], F32, tag="eo")
                    nc.vector.tensor_add(eo[:], ps[:, :2], b3bc[:, 2 * e:2 * e + 2])
                    if e == 0:
                        nc.vector.tensor_scalar_mul(acc[:, bt], eo[:],
                                                    exp_all[:, bt, 0:1])
                    else:
                        nc.vector.scalar_tensor_tensor(
                            acc[:, bt], eo[:], exp_all[:, bt, 1:2], acc[:, bt],
                            op0=ALU.mult, op1=ALU.add)
                        of = stpool.tile([128, 2], F32, tag="of")
                        nc.vector.tensor_scalar_mul(of[:], acc[:, bt],
                                                    recip_all[:, bt:bt + 1])
                        nc.sync.dma_start(out[bt * 128:(bt + 1) * 128, :], of[:])

            # -------- chunk-pipelined schedule --------
            # Expert-0 weight triggers go on the scalar stream right AFTER the
            # first chunk's LN sqrts (so the act-table load and sqrt are not
            # stuck behind the descriptor wall); expert-1 weights ride the sync
            # stream after all x-tile triggers. Gate logit matmuls are emitted
            # after the expert-0 w1 chains so the tensor queue never waits on
            # the (trigger-delayed) g1s RELU.
            ln_stage(0)
            # e0 weight triggers sit right after the first LN sqrt in the
            # scalar stream: early enough to overlap the LN/gate head, late
            # enough that the sqrt act-table DMA beats them through the queue.
            emit_w1(0, nc.scalar)
            emit_w2(0, nc.scalar)
            for bl in range(1, 2 * BPC):
                ln_stage(bl)
            # Transposes + gate chains for chunks 0-1 run while W1[e0] DMA
            # is still in flight, so expert-0 never waits on it.
            G1S = {}
            for cc in range(2):
                for bl in range(BPC):
                    xp_stage(cc * BPC + bl)
                G1S[cc] = gate_chains(cc)
            pend = None
            for c in range(NCH):
                if c >= 2:
                    for bl in range(BPC):
                        xp_stage(c * BPC + bl)
                    G1S[c] = gate_chains(c)
                w1_stage(0, c)
                gate_logits(c, G1S.pop(c))
                if pend is not None:
                    w3_stage(*pend)
                h2t = w2_stage(0, c)
                pend = (0, c, h2t)
                if c + 2 < NCH:
                    for bl in range(BPC):
                        ln_stage((c + 2) * BPC + bl)
            emit_w1(1, nc.sync)
            emit_w2(1, nc.sync)
            for c in range(NCH):
                w1_stage(1, c)
                if pend is not None:
                    w3_stage(*pend)
                h2t = w2_stage(1, c)
                pend = (1, c, h2t)
            w3_stage(*pend)

    nc.finalize()
    return nc


_NC_CACHE = None


def _get_nc():
    global _NC_CACHE
    if _NC_CACHE is None:
        _NC_CACHE = build_nc()
    return _NC_CACHE


def _shard_inputs(inputs):
    """Build the 8 per-core input maps: fold LN affines, pre-cast to bf16,
    pre-permute into the exact SBUF layouts (pure numpy)."""
    bf16 = ml_dtypes.bfloat16
    f = lambda a: np.asarray(a, dtype=np.float32)
    x = np.ascontiguousarray(f(inputs["x"])).astype(bf16)
    identity = np.eye(128, dtype=np.float32).astype(bf16)
    g_ln_g, g_ln_b = f(inputs["g_ln_g"]), f(inputs["g_ln_b"])
    g_w1, g_b1 = f(inputs["g_w1"]), f(inputs["g_b1"])
    g_w2, g_b2 = f(inputs["g_w2"]), f(inputs["g_b2"])
    e_ln_g, e_ln_b = f(inputs["e_ln_g"]), f(inputs["e_ln_b"])
    e_w1, e_b1 = f(inputs["e_w1"]), f(inputs["e_b1"])
    e_w2, e_b2 = f(inputs["e_w2"]), f(inputs["e_b2"])
    e_w3, e_b3 = f(inputs["e_w3"]), f(inputs["e_b3"])

    # Fold the layernorm affine through w1 (exact when beta is zero, which it
    # is for this model's inputs; the beta term folds into the bias).
    gw1f = g_w1 * g_ln_g[:, None]
    gb1f = g_b1 + g_ln_b @ g_w1
    ew1f = e_w1 * e_ln_g[:, :, None]
    eb1f = e_b1 + np.einsum("ei,eih->eh", e_ln_b, e_w1)

    gw1p = np.ascontiguousarray(
        gw1f.reshape(NK, 128, G1).transpose(1, 0, 2)).astype(bf16)
    gb1p = np.ascontiguousarray(gb1f.reshape(NG, 128).T)

    in_maps = []
    for c in range(NCORES):
        lo = c * EPL
        experts = list(range(lo, lo + EPL))
        # permute gate columns so this core's experts are columns 0..EPL-1
        perm = experts + [j for j in range(E) if j not in experts]
        gw2p = np.ascontiguousarray(
            g_w2[:, perm].reshape(NG, 128, E).transpose(1, 0, 2)).astype(bf16)
        w1p = np.ascontiguousarray(ew1f[experts].reshape(EPL * NK * 128, HID)
                                   ).astype(bf16)
        w2p = np.ascontiguousarray(e_w2[experts].reshape(EPL * KH * 128, G1)
                                   ).astype(bf16)
        w3p = np.ascontiguousarray(
            e_w3[experts].reshape(EPL, 2, 128, 2).transpose(2, 1, 0, 3)
            .reshape(128, 2 * EPL * 2)).astype(bf16)
        eb1p = np.ascontiguousarray(
            eb1f[experts].reshape(EPL, NM, 128).transpose(2, 0, 1))
        eb2p = np.ascontiguousarray(
            e_b2[experts].reshape(EPL, NG, 128).transpose(2, 0, 1))
        in_maps.append({
            "x": x,
            "identd": identity,
            "w1": w1p,
            "w2": w2p,
            "gw1": gw1p,
            "gw2": gw2p,
            "w3": w3p,
            "gb1": gb1p,
            "eb1": eb1p,
            "eb2": eb2p,
            "gb2": np.ascontiguousarray(g_b2[perm]),
            "eb3": np.ascontiguousarray(e_b3[experts]),
        })
    return in_maps


def _run(inputs, trace=False):
    nc = _get_nc()
    in_maps = _shard_inputs(inputs)
    res = run_bass_kernel_spmd(nc, in_maps, core_ids=list(range(NCORES)),
                               trace=trace)
    return res


def kernel(**inputs):
    res = _run(inputs, trace=bool(os.environ.get("MOE_TRACE")))
    total = np.zeros((B, 2), dtype=np.float64)
    for c in range(NCORES):
        total += res.results[c]["out"].astype(np.float64)
    pred_mean = total[:, 0:1].astype(np.float32)
    pv = np.logaddexp(0.0, total[:, 1:2]) + 1e-6
    pred_var = pv.astype(np.float32)
    kernel.last_exec_time_ns = getattr(res, "exec_time_ns", None)
    return pred_mean, pred_var


kernel.last_exec_time_ns = None


# revision 12
# speedup vs baseline: 1.5833x; 1.5833x over previous
"""MetaMoE Trainium2 kernel: 16 experts sharded 2-per-core across 8 NeuronCores.

Each core computes: shared LayerNorm of x, the (replicated) softmax gate, its two
experts' MLP chains, and the gate-weighted partial sum [B, 2]. The host sums the
8 partials and applies the final mean/var head split.

v2 layout strategy:
- All weights are folded (LN gain into w1) and cast to bf16 on the HOST, packed
  in the exact SBUF layout, so device DMA is contiguous and no on-device
  staging/cast pass is needed.
- x is normalized per 128-row tile, then transposed to feature-major via
  tensor-engine transposes (matmul-with-identity) into PSUM, copied to SBUF by
  the otherwise-idle GpSimd engine. This removes the serialized sync-ring
  DMA-transposes of v1 (and their ordering hazard against other DMA traffic).
- Chunk-pipelined schedule: per 512-batch chunk, LN+transpose -> gate -> expert0
  are emitted together so the tensor engine starts ~15us into the kernel;
  expert1 streams afterward from fully-resident inputs.
"""
import sys
import os

sys.path.insert(0, "/opt/trn_rl_repo")

import numpy as np
import ml_dtypes  # noqa: F401

import concourse.bass as bass  # noqa: F401
import concourse.mybir as mybir
from concourse import bacc
from concourse.tile import TileContext
from concourse.bass_utils import run_bass_kernel_spmd

F32 = mybir.dt.float32
BF16 = mybir.dt.bfloat16
AF = mybir.ActivationFunctionType
ALU = mybir.AluOpType
AX = mybir.AxisListType

B, IN, HID, G1, E = 4096, 1024, 2048, 256, 16
NCORES = 8
EPL = E // NCORES          # experts per core
NB = B // 128              # 32 batch tiles
NK = IN // 128             # 8 contraction tiles for w1 / gate w1
NM = HID // 128            # 16 m-tiles of h1
KH = HID // 128            # 16 contraction tiles for w2
NG = G1 // 128             # 2 m/k tiles for gate hidden
CH = 512                   # batch chunk (matmul moving free dim)
NCH = B // CH              # 8 chunks
BPC = CH // 128            # 4 b-tiles per chunk
EPS = 1e-5


def build_nc():
    nc = bacc.Bacc(None)

    x = nc.dram_tensor("x", [B, IN], BF16, kind="ExternalInput")
    identd = nc.dram_tensor("identd", [128, 128], BF16, kind="ExternalInput")
    w1 = nc.dram_tensor("w1", [EPL * NK * 128, HID], BF16, kind="ExternalInput")
    w2 = nc.dram_tensor("w2", [EPL * KH * 128, G1], BF16, kind="ExternalInput")
    gw1 = nc.dram_tensor("gw1", [128, NK, G1], BF16, kind="ExternalInput")
    gw2 = nc.dram_tensor("gw2", [128, NG, E], BF16, kind="ExternalInput")
    w3 = nc.dram_tensor("w3", [128, 2 * EPL * 2], BF16, kind="ExternalInput")
    gb1 = nc.dram_tensor("gb1", [128, NG], F32, kind="ExternalInput")
    eb1 = nc.dram_tensor("eb1", [128, EPL, NM], F32, kind="ExternalInput")
    eb2 = nc.dram_tensor("eb2", [128, EPL, NG], F32, kind="ExternalInput")
    gb2 = nc.dram_tensor("gb2", [E], F32, kind="ExternalInput")
    eb3 = nc.dram_tensor("eb3", [EPL, 2], F32, kind="ExternalInput")
    out = nc.dram_tensor("out", [B, 2], F32, kind="ExternalOutput")

    with TileContext(nc) as tc:
        with (
            tc.tile_pool(name="cpool", bufs=1) as cpool,
            tc.tile_pool(name="w1pool", bufs=EPL * NK) as w1pool,
            tc.tile_pool(name="w2pool", bufs=EPL * KH) as w2pool,
            tc.tile_pool(name="stage", bufs=2) as stpool,
            tc.tile_pool(name="xtpool", bufs=3) as xtpool,
            tc.tile_pool(name="xnbpool", bufs=8) as xnbpool,
            tc.tile_pool(name="hpool", bufs=1) as hpool,
            tc.tile_pool(name="psT", bufs=2, space="PSUM") as psT,
            tc.tile_pool(name="psA", bufs=2, space="PSUM") as psA,
            tc.tile_pool(name="psB", bufs=2, space="PSUM") as psB,
            tc.tile_pool(name="psC", bufs=2, space="PSUM") as psC,
        ):
            # ---------------- persistent tiles ----------------
            xnT = cpool.tile([128, NK, B], BF16)             # normalized x, transposed
            gw1b = cpool.tile([128, NK, G1], BF16)
            gw2b = cpool.tile([128, NG, E], BF16)
            w3b = cpool.tile([128, 2, EPL, 2], BF16)         # [p, k3, e, t]
            gb1_t = cpool.tile([128, NG], F32)
            b2bc = cpool.tile([128, E], F32)
            b3bc = cpool.tile([128, EPL * 2], F32)
            eb1_t = cpool.tile([128, EPL, NM], F32)
            eb2_t = cpool.tile([128, EPL, NG], F32)
            exp_all = cpool.tile([128, NB, E], F32)
            recip_all = cpool.tile([128, NB], F32)
            acc = cpool.tile([128, NB, 2], F32)
            ident = cpool.tile([128, 128], BF16)

            # Small constants ride the scalar queue so the sync queue's first
            # descriptors are the x tiles the LN pipeline is waiting on.
            nc.scalar.dma_start(ident[:], identd[:, :])
            nc.scalar.dma_start(gw1b[:], gw1[:, :, :])
            nc.scalar.dma_start(gw2b[:], gw2[:, :, :])
            nc.scalar.dma_start(
                w3b[:], w3.rearrange("p (k e t) -> p k e t", k=2, e=EPL))
            W1 = {}
            W2 = {}

            def emit_w1(e, eng):
                for k in range(NK):
                    t = w1pool.tile([128, HID], BF16, tag="w1")
                    r0 = (e * NK + k) * 128
                    eng.dma_start(t[:], w1[r0:r0 + 128, :])
                    W1[(e, k)] = t

            def emit_w2(e, eng):
                for k2 in range(KH):
                    t = w2pool.tile([128, G1], BF16, tag="w2")
                    r0 = (e * KH + k2) * 128
                    eng.dma_start(t[:], w2[r0:r0 + 128, :])
                    W2[(e, k2)] = t

            # ---------------- small constant loads (scalar ring) ---------------
            nc.scalar.dma_start(gb1_t[:], gb1[:, :])
            nc.scalar.dma_start(eb1_t[:], eb1[:, :, :])
            nc.scalar.dma_start(eb2_t[:], eb2[:, :, :])
            b2row = stpool.tile([1, E], F32, tag="b2row")
            nc.scalar.dma_start(b2row[:1, :], gb2[None, :])
            b3row = stpool.tile([1, EPL * 2], F32, tag="b3row")
            nc.scalar.dma_start(b3row[:1, :], eb3.rearrange("e t -> (e t)")[None, :])
            nc.gpsimd.partition_broadcast(b2bc[:], b2row[:1, :])
            nc.gpsimd.partition_broadcast(b3bc[:], b3row[:1, :])

            # ---------------- LayerNorm + tensor-engine transpose ------------
            XNB = {}

            def ln_stage(bt):
                xt = xtpool.tile([128, IN], BF16, tag="xt")
                nc.sync.dma_start(xt[:], x[bt * 128:(bt + 1) * 128, :])
                st6 = stpool.tile([128, 2, 6], F32, tag="st6")
                for g in range(2):
                    nc.vector.bn_stats(st6[:, g, :], xt[:, g * 512:(g + 1) * 512])
                st = stpool.tile([128, 2], F32, tag="st")
                nc.vector.bn_aggr(st[:], st6[:])
                rstd = stpool.tile([128, 1], F32, tag="rstd")
                nc.vector.tensor_scalar_add(rstd[:], st[:, 1:2], EPS)
                nc.scalar.sqrt(rstd[:], rstd[:])
                nc.vector.reciprocal(rstd[:], rstd[:])
                xnb = xnbpool.tile([128, IN], BF16, tag="xnb")
                nc.vector.tensor_scalar(xnb[:], xt[:], st[:, 0:1], rstd[:],
                                        op0=ALU.subtract, op1=ALU.mult)
                XNB[bt] = xnb

            def xp_stage(bt):
                xnb = XNB.pop(bt)
                ps = psT.tile([128, NK, 128], BF16, tag="psT")
                for k in range(NK):
                    nc.tensor.transpose(ps[:, k], xnb[:, k * 128:(k + 1) * 128],
                                        ident[:])
                nc.vector.tensor_copy(xnT[:, :, bt * 128:(bt + 1) * 128], ps[:])

            # ---------------- gate chunk routine ----------------
            def gate_chains(ch):
                c0 = ch * CH
                g1s = stpool.tile([128, NG, CH], BF16, tag="g1s")
                for m in range(NG):
                    ps = psB.tile([128, CH], F32, tag="psB")
                    for k in range(NK):
                        nc.tensor.matmul(ps[:], gw1b[:, k, m * 128:(m + 1) * 128],
                                         xnT[:, k, c0:c0 + CH],
                                         start=(k == 0), stop=(k == NK - 1))
                    nc.scalar.activation(g1s[:, m], ps[:], AF.Relu,
                                         bias=gb1_t[:, m:m + 1])
                return g1s

            def gate_logits(ch, g1s):
                for bl in range(BPC):
                    bt = ch * BPC + bl
                    ps = psC.tile([128, E], F32, tag="psC")
                    for k2 in range(NG):
                        nc.tensor.matmul(ps[:], g1s[:, k2, bl * 128:(bl + 1) * 128],
                                         gw2b[:, k2],
                                         start=(k2 == 0), stop=(k2 == NG - 1))
                    lg = stpool.tile([128, E], F32, tag="lg")
                    nc.vector.tensor_add(lg[:], ps[:], b2bc[:])
                    mx = stpool.tile([128, 1], F32, tag="mx")
                    nc.vector.tensor_reduce(mx[:], lg[:], axis=AX.X, op=ALU.max,
                                            negate=True)
                    sm = stpool.tile([128, 1], F32, tag="sm")
                    nc.scalar.activation(exp_all[:, bt], lg[:], AF.Exp,
                                         bias=mx[:, 0:1], accum_out=sm[:, 0:1])
                    nc.vector.reciprocal(recip_all[:, bt:bt + 1], sm[:])

            # ---------------- expert chunk routine ----------------
            h1s = hpool.tile([128, NM, CH], BF16)

            def w1_stage(e, ch):
                c0 = ch * CH
                for m in range(NM):
                    ps = psA.tile([128, CH], F32, tag="psA")
                    for k in range(NK):
                        nc.tensor.matmul(ps[:], W1[(e, k)][:, m * 128:(m + 1) * 128],
                                         xnT[:, k, c0:c0 + CH],
                                         start=(k == 0), stop=(k == NK - 1))
                    nc.scalar.activation(h1s[:, m], ps[:], AF.Relu,
                                         bias=eb1_t[:, e, m:m + 1])

            def w2_stage(e, ch):
                h2t = stpool.tile([128, NG, CH], BF16, tag="h2s")
                for m2 in range(NG):
                    ps = psB.tile([128, CH], F32, tag="psB")
                    for k2 in range(KH):
                        nc.tensor.matmul(ps[:], W2[(e, k2)][:, m2 * 128:(m2 + 1) * 128],
                                         h1s[:, k2],
                                         start=(k2 == 0), stop=(k2 == KH - 1))
                    nc.scalar.activation(h2t[:, m2], ps[:], AF.Relu,
                                         bias=eb2_t[:, e, m2:m2 + 1])
                return h2t

            def w3_stage(e, ch, h2t):
                for bl in range(BPC):
                    bt = ch * BPC + bl
                    ps = psC.tile([128, E], F32, tag="psC")
                    for k3 in range(2):
                        nc.tensor.matmul(ps[:, :2], h2t[:, k3, bl * 128:(bl + 1) * 128],
                                         w3b[:, k3, e],
                                         start=(k3 == 0), stop=(k3 == 1))
                    eo = stpool.tile([128, 2], F32, tag="eo")
                    nc.vector.tensor_add(eo[:], ps[:, :2], b3bc[:, 2 * e:2 * e + 2])
                    if e == 0:
                        nc.vector.tensor_scalar_mul(acc[:, bt], eo[:],
                                                    exp_all[:, bt, 0:1])
                    else:
                        nc.vector.scalar_tensor_tensor(
                            acc[:, bt], eo[:], exp_all[:, bt, 1:2], acc[:, bt],
                            op0=ALU.mult, op1=ALU.add)
                        of = stpool.tile([128, 2], F32, tag="of")
                        nc.vector.tensor_scalar_mul(of[:], acc[:, bt],
                                                    recip_all[:, bt:bt + 1])
                        nc.sync.dma_start(out[bt * 128:(bt + 1) * 128, :], of[:])

            # -------- chunk-pipelined schedule --------
            # Expert-0 weight triggers go on the scalar stream right AFTER the
            # first chunk's LN sqrts (so the act-table load and sqrt are not
            # stuck behind the descriptor wall); expert-1 weights ride the sync
            # stream after all x-tile triggers. Gate logit matmuls are emitted
            # after the expert-0 w1 chains so the tensor queue never waits on
            # the (trigger-delayed) g1s RELU.
            ln_stage(0)
            # e0 weight triggers sit right after the first LN sqrt in the
            # scalar stream: early enough to overlap the LN/gate head, late
            # enough that the sqrt act-table DMA beats them through the queue.
            emit_w1(0, nc.scalar)
            emit_w2(0, nc.scalar)
            for bl in range(1, 2 * BPC):
                ln_stage(bl)
            # Transposes + gate chains for chunks 0-1 run while W1[e0] DMA
            # is still in flight, so expert-0 never waits on it.
            G1S = {}
            for cc in range(2):
                for bl in range(BPC):
                    xp_stage(cc * BPC + bl)
                G1S[cc] = gate_chains(cc)
            pend = None
            for c in range(NCH):
                if c >= 2:
                    for bl in range(BPC):
                        xp_stage(c * BPC + bl)
                    G1S[c] = gate_chains(c)
                w1_stage(0, c)
                gate_logits(c, G1S.pop(c))
                if pend is not None:
                    w3_stage(*pend)
                h2t = w2_stage(0, c)
                pend = (0, c, h2t)
                if c + 2 < NCH:
                    for bl in range(BPC):
                        ln_stage((c + 2) * BPC + bl)
            emit_w1(1, nc.sync)
            emit_w2(1, nc.sync)
            for c in range(NCH):
                w1_stage(1, c)
                if pend is not None:
                    w3_stage(*pend)
                h2t = w2_stage(1, c)
                pend = (1, c, h2t)
            w3_stage(*pend)

    nc.finalize()
    return nc


_NC_CACHE = None


def _get_nc():
    global _NC_CACHE
    if _NC_CACHE is None:
        _NC_CACHE = build_nc()
    return _NC_CACHE


def _shard_inputs(inputs):
    """Build the 8 per-core input maps: fold LN affines, pre-cast to bf16,
    pre-permute into the exact SBUF layouts (pure numpy)."""
    bf16 = ml_dtypes.bfloat16
    f = lambda a: np.asarray(a, dtype=np.float32)
    x = np.ascontiguousarray(f(inputs["x"])).astype(bf16)
    identity = np.eye(128, dtype=np.float32).astype(bf16)
    g_ln_g, g_ln_b = f(inputs["g_ln_g"]), f(inputs["g_ln_b"])
    g_w1, g_b1 = f(inputs["g_w1"]), f(inputs["g_b1"])
    g_w2, g_b2 = f(inputs["g_w2"]), f(inputs["g_b2"])
    e_ln_g, e_ln_b = f(inputs["e_ln_g"]), f(inputs["e_ln_b"])
    e_w1, e_b1 = f(inputs["e_w1"]), f(inputs["e_b1"])
    e_w2, e_b2 = f(inputs["e_w2"]), f(inputs["e_b2"])
    e_w3, e_b3 = f(inputs["e_w3"]), f(inputs["e_b3"])

    # Fold the layernorm affine through w1 (exact when beta is zero, which it
    # is for this model's inputs; the beta term folds into the bias).
    gw1f = g_w1 * g_ln_g[:, None]
    gb1f = g_b1 + g_ln_b @ g_w1
    ew1f = e_w1 * e_ln_g[:, :, None]
    eb1f = e_b1 + np.einsum("ei,eih->eh", e_ln_b, e_w1)

    gw1p = np.ascontiguousarray(
        gw1f.reshape(NK, 128, G1).transpose(1, 0, 2)).astype(bf16)
    gb1p = np.ascontiguousarray(gb1f.reshape(NG, 128).T)

    in_maps = []
    for c in range(NCORES):
        lo = c * EPL
        experts = list(range(lo, lo + EPL))
        # permute gate columns so this core's experts are columns 0..EPL-1
        perm = experts + [j for j in range(E) if j not in experts]
        gw2p = np.ascontiguousarray(
            g_w2[:, perm].reshape(NG, 128, E).transpose(1, 0, 2)).astype(bf16)
        w1p = np.ascontiguousarray(ew1f[experts].reshape(EPL * NK * 128, HID)
                                   ).astype(bf16)
        w2p = np.ascontiguousarray(e_w2[experts].reshape(EPL * KH * 128, G1)
                                   ).astype(bf16)
        w3p = np.ascontiguousarray(
            e_w3[experts].reshape(EPL, 2, 128, 2).transpose(2, 1, 0, 3)
            .reshape(128, 2 * EPL * 2)).astype(bf16)
        eb1p = np.ascontiguousarray(
            eb1f[experts].reshape(EPL, NM, 128).transpose(2, 0, 1))
        eb2p = np.ascontiguousarray(
            e_b2[experts].reshape(EPL, NG, 128).transpose(2, 0, 1))
        in_maps.append({
            "x": x,
            "identd": identity,
            "w1": w1p,
            "w2": w2p,
            "gw1": gw1p,
            "gw2": gw2p,
            "w3": w3p,
            "gb1": gb1p,
            "eb1": eb1p,
            "eb2": eb2p,
            "gb2": np.ascontiguousarray(g_b2[perm]),
            "eb3": np.ascontiguousarray(e_b3[experts]),
        })
    return in_maps


def _run(inputs, trace=False):
    nc = _get_nc()
    in_maps = _shard_inputs(inputs)
    res = run_bass_kernel_spmd(nc, in_maps, core_ids=list(range(NCORES)),
                               trace=trace)
    return res


def kernel(**inputs):
    res = _run(inputs, trace=bool(os.environ.get("MOE_TRACE")))
    total = np.zeros((B, 2), dtype=np.float64)
    for c in range(NCORES):
        total += res.results[c]["out"].astype(np.float64)
    pred_mean = total[:, 0:1].astype(np.float32)
    pv = np.logaddexp(0.0, total[:, 1:2]) + 1e-6
    pred_var = pv.astype(np.float32)
    kernel.last_exec_time_ns = getattr(res, "exec_time_ns", None)
    return pred_mean, pred_var


kernel.last_exec_time_ns = None
